# revision 1
# baseline (speedup 1.0000x reference)
"""HSTU block kernel v4 for 8 TRN2 NeuronCores (nn_HSTU_66279935312625).

Sharding: 2 cores per batch (B=4). Core pair splits the 2048 rows
causally-balanced: core g=0 owns rows [0,512)+[1536,2048), g=1 owns
[512,1536). Every core recomputes K/V projections for all 2048 rows of
its batch (communication-free). The program is identical on all cores;
all per-core variation lives in host-prepared input data.

v4: generator-driven schedule. All matmul operands bf16 (PE weight
double-buffering), one [128,1024] sigmoid per 2-head kb step.
Emission order interleaves at kb-step granularity:
  phase1: K(h0) V(h0) K(h1) Q+U          (PE-bound, rope on Act/DVE/GpSimd)
  phase2a: attn(u0) chains with V(h1) groups as PE filler
  phase2b: attn(u1) chains with u0 stats/LN/out-proj as PE filler
  phase2c: u1 tail
Act sigmoid stream stays saturated; PE fills its slack with tail work.
"""
import numpy as np

import concourse.bacc as bacc
import concourse.tile as tile
from concourse import mybir
from concourse.bass_utils import run_bass_kernel_spmd

F32 = mybir.dt.float32
F32R = mybir.dt.float32r
BF16 = mybir.dt.bfloat16
AF = mybir.ActivationFunctionType
ALU = mybir.AluOpType

B, L, D, H, HD = 4, 2048, 1024, 16, 64
OWN = 1024          # rows owned per core
UQ = 512            # rows per q-unit
EXT = (8, 16)       # k-block extent per q-unit (128-row k blocks)
SCALE = HD ** -0.5
LN_EPS = 1e-8
NCORES = 8

_CACHED = {}


def _build():
    nc = bacc.Bacc("TRN2", target_bir_lowering=False, debug=False)

    xkvT = nc.dram_tensor("xkvT", [D, L], BF16, kind="ExternalInput").ap()
    xqT = nc.dram_tensor("xqT", [D, OWN], BF16, kind="ExternalInput").ap()
    xq = nc.dram_tensor("xq", [OWN, D], F32, kind="ExternalInput").ap()
    wproj = nc.dram_tensor("wproj", [D, 4 * D], BF16, kind="ExternalInput").ap()
    wout = nc.dram_tensor("wout", [D, D], BF16, kind="ExternalInput").ap()
    cosk = nc.dram_tensor("cosk", [128, L], BF16, kind="ExternalInput").ap()
    sink = nc.dram_tensor("sink", [128, L], BF16, kind="ExternalInput").ap()
    cosq = nc.dram_tensor("cosq", [128, OWN], BF16, kind="ExternalInput").ap()
    sinq = nc.dram_tensor("sinq", [128, OWN], BF16, kind="ExternalInput").ap()
    p2 = nc.dram_tensor("p2", [128, 128], BF16, kind="ExternalInput").ap()
    maskT = nc.dram_tensor("maskT", [128, 16, UQ], BF16,
                           kind="ExternalInput").ap()
    bprojT = nc.dram_tensor("bprojT", [128, 32], F32, kind="ExternalInput").ap()
    gamT = nc.dram_tensor("gamT", [128, 8], F32R, kind="ExternalInput").ap()
    betT = nc.dram_tensor("betT", [128, 8], F32R, kind="ExternalInput").ap()
    bcol = nc.dram_tensor("bcol", [1, 4 * D], BF16, kind="ExternalInput").ap()
    ones128 = nc.dram_tensor("ones128", [128, 1], BF16,
                             kind="ExternalInput").ap()
    onesrow = nc.dram_tensor("onesrow", [1, UQ], BF16,
                             kind="ExternalInput").ap()
    onesrowF = nc.dram_tensor("onesrowF", [1, 128], F32R,
                              kind="ExternalInput").ap()
    vbias = nc.dram_tensor("vbias", [128, D], BF16, kind="ExternalInput").ap()
    out = nc.dram_tensor("out", [OWN, D], F32, kind="ExternalOutput").ap()

    wp3 = wproj.rearrange("(t ki) n -> ki t n", ki=128)   # [128, 8, 4096]
    wo3 = wout.rearrange("(t ki) n -> ki t n", ki=128)    # [128, 8, 1024]
    xkv3 = xkvT.rearrange("(t ki) n -> ki t n", ki=128)   # [128, 8, 2048]
    xq3 = xqT.rearrange("(t ki) n -> ki t n", ki=128)     # [128, 8, 1024]

    with tile.TileContext(nc) as tc:
        with (
            tc.tile_pool(name="const", bufs=1) as cpool,
            tc.tile_pool(name="big", bufs=1) as big,
            tc.tile_pool(name="ph1x1", bufs=1) as ph1x1,
            tc.tile_pool(name="wring", bufs=2) as wring,
        ):
            ph1scope = [tc.tile_pool(name="rope", bufs=3),
                        tc.tile_pool(name="trig", bufs=1),
                        tc.tile_pool(name="ppj", bufs=6, space="PSUM"),
                        tc.tile_pool(name="prt", bufs=2, space="PSUM")]
            rope, trig, ppj, prt = [p.__enter__() for p in ph1scope]
            ph1pool = tc.tile_pool(name="ph1", bufs=1)
            ph1 = ph1pool.__enter__()
            # ---- first-needed DMAs first ----
            wk0 = wring.tile([128, 8, 128], BF16, tag="wk")
            nc.sync.dma_start(wk0[:], wp3[:, :, 3 * D:3 * D + 128])
            p2sb = cpool.tile([128, 128], BF16)
            nc.sync.dma_start(p2sb[:], p2)
            bprojsb = cpool.tile([128, 32], F32)
            nc.sync.dma_start(bprojsb[:], bprojT)
            cksb0 = trig.tile([128, 1024], BF16, tag="cksb")
            nc.sync.dma_start(cksb0[:], cosk[:, 0:1024])
            sksb0 = trig.tile([128, 1024], BF16, tag="sksb")
            nc.sync.dma_start(sksb0[:], sink[:, 0:1024])
            xh0 = ph1.tile([128, 8, 1024], BF16, tag="xh")
            for t in range(8):
                nc.sync.dma_start(xh0[:, t, :], xkv3[:, t, 0:1024])
            gamsb = cpool.tile([128, 8], F32R)
            nc.sync.dma_start(gamsb[:], gamT)
            betsb = cpool.tile([128, 8], F32R)
            nc.sync.dma_start(betsb[:], betT)
            o128 = cpool.tile([128, 1], BF16)
            nc.sync.dma_start(o128[:], ones128)
            orow = cpool.tile([1, UQ], BF16)
            nc.sync.dma_start(orow[:], onesrow)
            orowF = cpool.tile([1, 128], F32R)
            nc.sync.dma_start(orowF[:], onesrowF)
            vbsb = cpool.tile([128, D], BF16)
            nc.sync.dma_start(vbsb[:], vbias)
            epsb = cpool.tile([1, 1], F32)
            nc.vector.memset(epsb[:], LN_EPS)

            krot = big.tile([128, 8, L], BF16)      # K_rot^T
            v16 = big.tile([128, 16, D], BF16)      # V natural [row tiles]
            qrot = big.tile([128, 8, OWN], BF16)    # Q_rot^T
            silu16 = big.tile([128, 8, OWN], BF16)  # silu(U)^T

            def rope_chain(psP, bias, cos_sl, sin_sl, dst):
                # dst(bf16) = (psP+b)*cos + rotate_half(psP+b)*sin
                t16 = rope.tile([128, UQ], BF16, tag="t16")
                nc.scalar.activation(t16[:], psP[:], AF.Identity, bias=bias)
                psR = prt.tile([128, UQ], F32, tag="psR")
                nc.tensor.matmul(psR[:], p2sb[:], t16[:], start=True,
                                 stop=True)
                tcos = rope.tile([128, UQ], F32, tag="tcos")
                nc.vector.scalar_tensor_tensor(
                    tcos[:], psP[:], bias, cos_sl, ALU.add, ALU.mult)
                tsin = rope.tile([128, UQ], F32, tag="tsin")
                nc.vector.tensor_mul(tsin[:], psR[:], sin_sl)
                nc.gpsimd.tensor_add(dst, tcos[:], tsin[:])

            def k_block(h, xh, cksb, sksb, first_wk=None):
                for ct in range(8):
                    c0 = 3 * D + 128 * ct
                    if first_wk is not None and ct == 0:
                        wk = first_wk
                    else:
                        wk = wring.tile([128, 8, 128], BF16, tag="wk")
                        nc.sync.dma_start(wk[:], wp3[:, :, c0:c0 + 128])
                    for r in range(2):
                        ps = ppj.tile([128, UQ], F32, tag="ps")
                        for t in range(8):
                            nc.tensor.matmul(
                                ps[:], wk[:, t, :],
                                xh[:, t, r * UQ:(r + 1) * UQ],
                                start=(t == 0), stop=(t == 7))
                        off = r * UQ
                        rope_chain(ps, bprojsb[:, 24 + ct:25 + ct],
                                   cksb[:, off:off + UQ],
                                   sksb[:, off:off + UQ],
                                   krot[:, ct, h * 1024 + off:
                                        h * 1024 + off + UQ])

            def v_group_gen(h, xh, vpool):
                for vh in range(2):
                    v0 = D + UQ * vh
                    wvh = wring.tile([128, 8, UQ], BF16, tag="wv", bufs=1)
                    nc.sync.dma_start(wvh[:], wp3[:, :, v0:v0 + UQ])
                    for rv in range(8):
                        grv = h * 8 + rv
                        pv = vpool.tile([128, UQ], F32, tag="ps")
                        for t in range(8):
                            nc.tensor.matmul(
                                pv[:], xh[:, t, 128 * rv:128 * (rv + 1)],
                                wvh[:, t, :], start=(t == 0), stop=(t == 7))
                            if t == 3:
                                yield
                        nc.vector.scalar_tensor_tensor(
                            v16[:, grv, UQ * vh:UQ * (vh + 1)], pv[:], 0.0,
                            vbsb[:, UQ * vh:UQ * (vh + 1)],
                            ALU.add, ALU.add)
                        yield

            # ---------- phase 1: K(h0), V(h0), K(h1), Q+U ----------
            xh1 = ph1x1.tile([128, 8, 1024], BF16, tag="xh1")
            nc.sync.dma_start(xh1[:], xkv3[:, :, 1024:2048])
            k_block(0, xh0, cksb0, sksb0, first_wk=wk0)
            for _ in v_group_gen(0, xh0, ppj):
                pass
            ph1pool.__exit__(None, None, None)
            cksb1 = trig.tile([128, 1024], BF16, tag="cksb")
            nc.sync.dma_start(cksb1[:], cosk[:, 1024:2048])
            sksb1 = trig.tile([128, 1024], BF16, tag="sksb")
            nc.sync.dma_start(sksb1[:], sink[:, 1024:2048])
            k_block(1, xh1, cksb1, sksb1)

            with (
                tc.tile_pool(name="ph1a", bufs=1) as ph1a,
                tc.tile_pool(name="wring2", bufs=2) as wring2,
            ):
                xqsb = ph1a.tile([128, 8, OWN], BF16)
                nc.sync.dma_start(xqsb[:], xq3)
                cqsb = ph1a.tile([128, OWN], BF16)
                nc.sync.dma_start(cqsb[:], cosq)
                sqsb = ph1a.tile([128, OWN], BF16)
                nc.sync.dma_start(sqsb[:], sinq)

                for ct in range(8):
                    wu = wring2.tile([128, 8, 128], BF16, tag="wu")
                    nc.sync.dma_start(wu[:], wp3[:, :, 128 * ct:128 * (ct + 1)])
                    q0 = 2 * D + 128 * ct
                    wq = wring2.tile([128, 8, 128], BF16, tag="wq")
                    nc.sync.dma_start(wq[:], wp3[:, :, q0:q0 + 128])
                    for r in range(2):
                        sl = slice(r * UQ, (r + 1) * UQ)
                        psu = ppj.tile([128, UQ], F32, tag="ps")
                        for t in range(8):
                            nc.tensor.matmul(psu[:], wu[:, t, :],
                                             xqsb[:, t, sl],
                                             start=(t == 0), stop=(t == 7))
                        nc.scalar.activation(silu16[:, ct, sl], psu[:],
                                             AF.Silu,
                                             bias=bprojsb[:, ct:ct + 1])
                        psq = ppj.tile([128, UQ], F32, tag="ps")
                        for t in range(8):
                            nc.tensor.matmul(psq[:], wq[:, t, :],
                                             xqsb[:, t, sl],
                                             start=(t == 0), stop=(t == 7))
                        rope_chain(psq, bprojsb[:, 16 + ct:17 + ct],
                                   cqsb[:, sl], sqsb[:, sl],
                                   qrot[:, ct, sl])

            for p in reversed(ph1scope):
                p.__exit__(None, None, None)

            # ---------- phase 2 ----------
            with (
                tc.tile_pool(name="ph2", bufs=1) as ph2,
                tc.tile_pool(name="aring", bufs=3) as aring,
                tc.tile_pool(name="sqring", bufs=1) as sqring,
                tc.tile_pool(name="bcast", bufs=1) as bcast,
                tc.tile_pool(name="gring", bufs=1) as gring,
                tc.tile_pool(name="oring", bufs=2) as oring,
                tc.tile_pool(name="woring", bufs=1) as woring,
            ):
                attn_scope = [tc.tile_pool(name="psS_", bufs=2, space="PSUM"),
                              tc.tile_pool(name="psO_", bufs=2, space="PSUM")]
                psSp, psOp = [p.__enter__() for p in attn_scope]
                attnT = ph2.tile([128, 8, L // 2], BF16)
                statr = ph2.tile([1, 4, UQ], F32R)
                tail_sb = {}

                def attn_chain_gen(u, hp):
                    psO = psOp.tile([128, UQ], F32, tag="psO")
                    qA = qrot[0:64, hp, u * UQ:(u + 1) * UQ]
                    qB = qrot[64:128, hp, u * UQ:(u + 1) * UQ]
                    kbs = range(EXT[u] - 1, -1, -1) if u == 1 else \
                        range(EXT[u])
                    for j, kb in enumerate(kbs):
                        psS = psSp.tile([128, 1024], F32, tag="psS")
                        nc.tensor.matmul(
                            psS[:, 0:UQ],
                            krot[0:64, hp, 128 * kb:128 * (kb + 1)],
                            qA, start=True, stop=True)
                        nc.tensor.matmul(
                            psS[:, UQ:1024],
                            krot[64:128, hp, 128 * kb:128 * (kb + 1)],
                            qB, start=True, stop=True)
                        aAB = aring.tile([128, 1024], BF16, tag="aAB")
                        nc.scalar.activation(aAB[:], psS[:], AF.Sigmoid,
                                             scale=SCALE)
                        mi = kb - 8 * u
                        if 0 <= mi < 8:
                            mj = 8 * u + mi
                            nc.vector.tensor_mul(aAB[:, 0:UQ], aAB[:, 0:UQ],
                                                 msb[:, mj, :])
                            nc.vector.tensor_mul(aAB[:, UQ:1024],
                                                 aAB[:, UQ:1024],
                                                 msb[:, mj, :])
                        last = j == EXT[u] - 1
                        nc.tensor.matmul(
                            psO[0:64, :],
                            v16[:, kb, 128 * hp:128 * hp + 64],
                            aAB[:, 0:UQ], start=(j == 0), stop=last,
                            tile_position=(0, 0))
                        nc.tensor.matmul(
                            psO[64:128, :],
                            v16[:, kb, 128 * hp + 64:128 * (hp + 1)],
                            aAB[:, UQ:1024], start=(j == 0), stop=last,
                            tile_position=(0, 64))
                        yield
                    nc.vector.tensor_copy(
                        attnT[:, hp, u * UQ:(u + 1) * UQ], psO[:])

                def stats_gen(u):
                    usl = slice(u * UQ, (u + 1) * UQ)
                    with tc.tile_pool(name=f"psT{u}", bufs=2,
                                      space="PSUM") as psTp:
                        psSum = psTp.tile([1, UQ], F32, tag="st")
                        psSq = psTp.tile([1, UQ], F32, tag="st")
                        for hp in range(8):
                            sq = sqring.tile([128, UQ], BF16, tag="sq")
                            nc.vector.tensor_mul(sq[:], attnT[:, hp, usl],
                                                 attnT[:, hp, usl])
                            nc.tensor.matmul(psSum[:], o128[:],
                                             attnT[:, hp, usl],
                                             start=(hp == 0), stop=(hp == 7))
                            nc.tensor.matmul(psSq[:], o128[:], sq[:],
                                             start=(hp == 0), stop=(hp == 7))
                            yield

                        mu = statr[0:1, 0, :]
                        nc.vector.tensor_scalar_mul(mu, psSum[:], 1.0 / D)
                        m2 = statr[0:1, 1, :]
                        nc.vector.tensor_scalar_mul(m2, psSq[:], 1.0 / D)
                        musq = statr[0:1, 2, :]
                        nc.vector.tensor_mul(musq, mu, mu)
                        varr = statr[0:1, 1, :]
                        nc.vector.tensor_sub(varr, m2, musq)
                        rstd = statr[0:1, 3, :]
                        nc.scalar.activation(rstd, varr, AF.Sqrt, bias=epsb[:])
                        with nc.allow_low_precision("f32r rstd for matmul"):
                            nc.vector.reciprocal(rstd, rstd)
                        nmr = statr[0:1, 2, :]
                        nc.vector.tensor_mul(nmr, mu, rstd)
                        nc.vector.tensor_scalar_mul(nmr, nmr, -1.0)
                        yield

                def ln_gen(u):
                    usl = slice(u * UQ, (u + 1) * UQ)
                    with tc.tile_pool(name=f"psG{u}", bufs=2,
                                      space="PSUM") as psGp:
                        psRb = psGp.tile([128, UQ], F32, tag="bc")
                        nc.tensor.matmul(psRb[:], orowF[0:1, :],
                                         statr[0:1, 3, :],
                                         start=True, stop=True)
                        rstd_b = bcast.tile([128, UQ], BF16, tag="rb")
                        nc.scalar.activation(rstd_b[:], psRb[:], AF.Copy)
                        psNb = psGp.tile([128, UQ], F32, tag="bc")
                        nc.tensor.matmul(psNb[:], orowF[0:1, :],
                                         statr[0:1, 2, :],
                                         start=True, stop=True)
                        nmr_b = bcast.tile([128, UQ], BF16, tag="nb")
                        nc.scalar.activation(nmr_b[:], psNb[:], AF.Copy)
                        yield

                        # gated = ((aT*gam)*rstd_b + (nmr_b*gam+bet))*silu
                        for c in range(8):
                            g1 = gring.tile([128, UQ], BF16, tag="g1")
                            nc.vector.scalar_tensor_tensor(
                                g1[:], attnT[:, c, usl], gamsb[:, c:c + 1],
                                rstd_b[:], ALU.mult, ALU.mult)
                            g2 = gring.tile([128, UQ], BF16, tag="g2")
                            nc.vector.scalar_tensor_tensor(
                                g2[:], nmr_b[:], gamsb[:, c:c + 1], g1[:],
                                ALU.mult, ALU.add)
                            nc.vector.scalar_tensor_tensor(
                                attnT[:, c, usl], g2[:], betsb[:, c:c + 1],
                                silu16[:, c, usl],
                                ALU.add, ALU.mult)
                            yield

                def outproj_gen(u, oh):
                    wo = woring.tile([128, 8, UQ], BF16, tag="wo")
                    nc.sync.dma_start(wo[:], wo3[:, :, UQ * oh:UQ * (oh + 1)])
                    with tc.tile_pool(name=f"psP{u}{oh}", bufs=2,
                                      space="PSUM") as psPp:
                        for rw in range(4):
                            r0 = u * UQ + 128 * rw
                            xqn = oring.tile([128, UQ], F32, tag="xqn", bufs=1)
                            nc.sync.dma_start(
                                xqn[:], xq[r0:r0 + 128,
                                           UQ * oh:UQ * (oh + 1)])
                            psOut = psPp.tile([128, UQ], F32, tag="po")
                            for c in range(8):
                                st = attnT[:, c,
                                           u * UQ + 128 * rw:
                                           u * UQ + 128 * (rw + 1)]
                                nc.tensor.matmul(
                                    psOut[:], st, wo[:, c, :],
                                    start=(c == 0), stop=(c == 7))
                                if c == 3:
                                    yield
                            osb = oring.tile([128, UQ], F32, tag="osb")
                            nc.vector.tensor_add(osb[:], psOut[:], xqn[:])
                            nc.sync.dma_start(
                                out[r0:r0 + 128, UQ * oh:UQ * (oh + 1)],
                                osb[:])
                            yield

                def drive(chains, fillers, steps_per=1):
                    """Round-robin: per chain step, advance filler."""
                    fi = iter(fillers)
                    cur = None
                    for ch in chains:
                        for _ in ch:
                            for _ in range(steps_per):
                                if cur is None:
                                    cur = next(fi, None)
                                    if cur is None:
                                        break
                                if next(cur, StopIteration) is StopIteration:
                                    cur = None
                    # drain remaining fillers
                    while True:
                        if cur is None:
                            cur = next(fi, None)
                            if cur is None:
                                break
                        if next(cur, StopIteration) is StopIteration:
                            cur = None

                # 2a: u0 chains, V(h1) as filler (u0 needs only kb<8)
                msb = ph2.tile([128, 16, UQ], BF16)
                nc.sync.dma_start(msb[:], maskT)
                with tc.tile_pool(name="pvj", bufs=2, space="PSUM") as pvj:
                    drive([attn_chain_gen(0, hp) for hp in range(8)]
                          + [attn_chain_gen(1, 0)],
                          [v_group_gen(1, xh1, pvj)])
                # 2b: u1 chains, u0 tail as filler
                drive([attn_chain_gen(1, hp) for hp in range(1, 8)],
                      [stats_gen(0), ln_gen(0),
                       outproj_gen(0, 0), outproj_gen(0, 1)])
                for p in reversed(attn_scope):
                    p.__exit__(None, None, None)

                # 2c: u1 tail, fused ln+outproj (c-outer, 8 PSUM banks)
                u = 1
                usl = slice(u * UQ, (u + 1) * UQ)
                for _ in stats_gen(u):
                    pass
                with tc.tile_pool(name="psG1f", bufs=2,
                                  space="PSUM") as psGp:
                    psRb = psGp.tile([128, UQ], F32, tag="bc")
                    nc.tensor.matmul(psRb[:], orowF[0:1, :],
                                     statr[0:1, 3, :], start=True, stop=True)
                    rstd_b = bcast.tile([128, UQ], BF16, tag="rb")
                    nc.scalar.activation(rstd_b[:], psRb[:], AF.Copy)
                    psNb = psGp.tile([128, UQ], F32, tag="bc")
                    nc.tensor.matmul(psNb[:], orowF[0:1, :],
                                     statr[0:1, 2, :], start=True, stop=True)
                    nmr_b = bcast.tile([128, UQ], BF16, tag="nb")
                    nc.scalar.activation(nmr_b[:], psNb[:], AF.Copy)
                wo0 = woring.tile([128, 8, UQ], BF16, tag="wo")
                nc.sync.dma_start(wo0[:], wo3[:, :, 0:UQ])
                wo1 = woring.tile([128, 8, UQ], BF16, tag="wo1")
                nc.sync.dma_start(wo1[:], wo3[:, :, UQ:D])
                wos = (wo0, wo1)
                with tc.tile_pool(name="psPF", bufs=8,
                                  space="PSUM") as psPp:
                    psOut = [psPp.tile([128, UQ], F32, tag="po",
                                       name=f"pof{i}")
                             for i in range(8)]
                    for c in range(8):
                        g1 = gring.tile([128, UQ], BF16, tag="g1")
                        nc.vector.scalar_tensor_tensor(
                            g1[:], attnT[:, c, usl], gamsb[:, c:c + 1],
                            rstd_b[:], ALU.mult, ALU.mult)
                        g2 = gring.tile([128, UQ], BF16, tag="g2")
                        nc.vector.scalar_tensor_tensor(
                            g2[:], nmr_b[:], gamsb[:, c:c + 1], g1[:],
                            ALU.mult, ALU.add)
                        nc.vector.scalar_tensor_tensor(
                            attnT[:, c, usl], g2[:], betsb[:, c:c + 1],
                            silu16[:, c, usl], ALU.add, ALU.mult)
                        for rw in range(4):
                            st = attnT[:, c,
                                       u * UQ + 128 * rw:
                                       u * UQ + 128 * (rw + 1)]
                            for oh in range(2):
                                nc.tensor.matmul(
                                    psOut[rw * 2 + oh][:], st,
                                    wos[oh][:, c, :],
                                    start=(c == 0), stop=(c == 7))
                    for rw in range(4):
                        r0 = u * UQ + 128 * rw
                        xqn = oring.tile([128, D], F32, tag="xqf", bufs=1)
                        nc.sync.dma_start(xqn[:], xq[r0:r0 + 128, :])
                        for oh in range(2):
                            osb = oring.tile([128, UQ], F32, tag="osb")
                            nc.vector.tensor_add(
                                osb[:], psOut[rw * 2 + oh][:],
                                xqn[:, UQ * oh:UQ * (oh + 1)])
                            nc.sync.dma_start(
                                out[r0:r0 + 128, UQ * oh:UQ * (oh + 1)],
                                osb[:])
    nc.finalize()
    return nc


def _host_prep(x, attn_mask, W_proj, b_proj, ln_gamma, ln_beta, W_out, b_out):
    """Build the 8 per-core input maps."""
    import ml_dtypes
    bf16 = ml_dtypes.bfloat16

    x = np.asarray(x, dtype=np.float32)
    attn_mask = np.asarray(attn_mask)
    W_proj = np.ascontiguousarray(np.asarray(W_proj, dtype=np.float32))
    W_out = np.ascontiguousarray(np.asarray(W_out, dtype=np.float32))
    b_proj = np.asarray(b_proj, dtype=np.float32)
    b_out = np.asarray(b_out, dtype=np.float32)
    ln_gamma = np.asarray(ln_gamma, dtype=np.float32)
    ln_beta = np.asarray(ln_beta, dtype=np.float32)

    inv = 1.0 / (10000.0 ** (np.arange(0, HD, 2, dtype=np.float64) / HD))
    ang = np.outer(inv, np.arange(L, dtype=np.float64))       # [32, L]
    c64 = np.concatenate([np.cos(ang), np.cos(ang)], 0)
    s64 = np.concatenate([np.sin(ang), np.sin(ang)], 0)
    cosk = np.concatenate([c64, c64], 0).astype(np.float32)   # [128, L]
    sink = np.concatenate([s64, s64], 0).astype(np.float32)

    p2 = np.zeros((128, 128), dtype=np.float32)
    for base in (0, 64):
        for m in range(32):
            p2[base + m + 32, base + m] = -1.0
        for m in range(32, 64):
            p2[base + m - 32, base + m] = 1.0

    shared = dict(
        wproj=W_proj.astype(bf16), wout=W_out.astype(bf16),
        cosk=cosk.astype(bf16), sink=sink.astype(bf16),
        p2=p2.astype(bf16),
        bprojT=np.ascontiguousarray(b_proj.reshape(32, 128).T),
        gamT=np.ascontiguousarray(ln_gamma.reshape(8, 128).T),
        betT=np.ascontiguousarray(ln_beta.reshape(8, 128).T),
        bcol=b_proj.reshape(1, 4 * D).astype(bf16),
        ones128=np.ones((128, 1), np.float32).astype(bf16),
        onesrow=np.ones((1, UQ), np.float32).astype(bf16),
        onesrowF=np.ones((1, 128), np.float32),
        vbias=np.broadcast_to(b_proj[D:2 * D], (128, D)).astype(bf16),
    )

    in_maps = []
    for c in range(NCORES):
        b, g = divmod(c, 2)
        own = np.r_[0:512, 1536:2048] if g == 0 else np.r_[512:1536]
        xb = x[b]
        xqc = np.ascontiguousarray(xb[own])
        m = dict(shared)
        m["xkvT"] = np.ascontiguousarray(xb.T).astype(bf16)
        m["xqT"] = np.ascontiguousarray(xqc.T).astype(bf16)
        m["xq"] = xqc + b_out[None, :]
        m["cosq"] = np.ascontiguousarray(cosk[:, own]).astype(bf16)
        m["sinq"] = np.ascontiguousarray(sink[:, own]).astype(bf16)
        mk = np.zeros((16, 128, UQ), dtype=np.float32)
        am = attn_mask[b]
        for u in range(2):
            qg = own[u * UQ:(u + 1) * UQ]
            for kb in range(8 * u, 8 * u + 8):
                mk[kb] = am[qg][:, 128 * kb:128 * (kb + 1)].T
        m["maskT"] = np.ascontiguousarray(
            mk.transpose(1, 0, 2).astype(bf16))
        in_maps.append(m)
    return in_maps


def kernel(**inputs):
    if "nc" not in _CACHED:
        _CACHED["nc"] = _build()
    nc = _CACHED["nc"]
    in_maps = _host_prep(**inputs)
    res = run_bass_kernel_spmd(nc, in_maps, list(range(NCORES)))
    full = np.empty((B, L, D), dtype=np.float32)
    for c in range(NCORES):
        b, g = divmod(c, 2)
        o = res.results[c]["out"]
        if g == 0:
            full[b, 0:512] = o[0:512]
            full[b, 1536:2048] = o[512:1024]
        else:
            full[b, 512:1536] = o
    return full

